# revision 4
# baseline (speedup 1.0000x reference)
"""Causal self-attention (B=1, S=4096, C=1024, NH=16) on 8 Trainium2
NeuronCores.

Sharding: heads 2-per-core (tensor parallel). Wqkv column-sharded,
Wo row-sharded; each core computes a full-shape partial of the output
projection and the host sums the 8 partials (+ Wo bias).

Per-core dataflow (all matmuls in float32r — fp32 storage, tf32-class
matmul precision at full PE rate):
  xT (C on partitions, host-pretransposed) -> qT/kT [128=2*64hd, S] and
  v [S, hd+ones] via the QKV projection; flash-style attention with
  k-major score tiles scoresT[sk,sq] so softmax denominators ride the
  PV matmul as an appended ones column of V; exp straight from PSUM on
  ScalarE; causal masking via an additive sliding-window mask on the
  diagonal k-blocks; out^T = v_aug.T @ exp(scoresT) accumulates in PSUM
  [65, span]; normalization via reciprocal + K=1 broadcast matmul; the
  output projection consumes attn^T directly and partial y rows DMA
  from PSUM to DRAM.
"""
import sys

sys.path.insert(0, "/opt/trn_rl_repo")

import numpy as np

import concourse.bass as bass
import concourse.mybir as mybir
from concourse import tile

F32 = mybir.dt.float32
F32R = mybir.dt.float32r

S = 4096
C = 1024
NH = 16
HD = 64
NCORES = 8
HPC = NH // NCORES          # heads per core = 2
J = HPC * HD                # 128 qkv rows per section per core
SPAN = 512                  # q-span / s-chunk
NSPAN = S // SPAN           # 8
KB = 128                    # k-block
NEG = -1.0e9


# ---------------------------------------------------------------- fixups
def _split_waits(nc, max_waits=1):
    """This container's walrus rejects >1 sync-wait on several instruction
    structs; hoist excess waits onto single-wait EventSemaphore carriers
    inserted just before the instruction (same engine)."""
    wid = 0
    for f in nc.m.functions:
        for bb in f.blocks:
            insts = bb.instructions
            i = 0
            while i < len(insts):
                ins = insts[i]
                si = getattr(ins, "sync_info", None)
                if si is not None and len(si.on_wait) > max_waits:
                    waits = list(si.on_wait)
                    si.on_wait = waits[:max_waits]
                    for w in waits[max_waits:]:
                        wid += 1
                        insts.insert(i, mybir.InstEventSemaphore(
                            name=f"WSPLIT-{wid}",
                            engine=ins.engine,
                            ins=[], outs=[],
                            sync_info=mybir.SyncInfo(on_wait=[w], on_update=[]),
                        ))
                        i += 1
                i += 1


# ---------------------------------------------------------------- program
def build_nc(reps: int = 1) -> bass.Bass:
    nc = bass.Bass()
    xT = nc.dram_tensor("xT", [C, S], F32R, kind="ExternalInput")
    wqk = nc.dram_tensor("wqk", [C, 2 * J], F32R, kind="ExternalInput")
    wv = nc.dram_tensor("wv", [C, J], F32R, kind="ExternalInput")
    wo = nc.dram_tensor("wo", [J, C], F32R, kind="ExternalInput")
    bqk = nc.dram_tensor("bqk", [J, 2], F32, kind="ExternalInput")
    bv = nc.dram_tensor("bv", [1, J], F32, kind="ExternalInput")
    mbig = nc.dram_tensor("mbig", [KB, 896], F32, kind="ExternalInput")
    onesd = nc.dram_tensor("onesd", [1, 65], F32R, kind="ExternalInput")
    y = nc.dram_tensor("y", [S, C], F32, kind="ExternalOutput")

    with tile.TileContext(nc) as tc:
        with (
            nc.allow_low_precision(reason="f32r is full-rate on PE; rounding error is acceptable here"),
            tc.tile_pool(name="const", bufs=1) as constp,
            tc.tile_pool(name="persist", bufs=1) as persist,
            tc.tile_pool(name="xp", bufs=2) as xp,
            tc.tile_pool(name="ptp", bufs=4) as ptp,
            tc.tile_pool(name="attnp", bufs=2) as attnp,
            tc.tile_pool(name="up", bufs=2) as up,
            tc.tile_pool(name="yp", bufs=4) as ypool,
            tc.tile_pool(name="rcp", bufs=2) as rcp,
            tc.tile_pool(name="scores", bufs=2, space="PSUM") as scoresp,
            tc.tile_pool(name="outtp", bufs=2, space="PSUM") as outtp,
            tc.tile_pool(name="mmp", bufs=2, space="PSUM") as mmp,
        ):
            # ---- constants ----
            wqk_sb = constp.tile([128, 8, 2 * J], F32R, tag="wqk")
            nc.sync.dma_start(out=wqk_sb, in_=wqk.rearrange("(a p) j -> p a j", p=128))
            wv_sb = constp.tile([128, 8, J], F32R, tag="wv")
            nc.sync.dma_start(out=wv_sb, in_=wv.rearrange("(a p) j -> p a j", p=128))
            wo_sb = constp.tile([J, C], F32R, tag="wo")
            nc.sync.dma_start(out=wo_sb, in_=wo[:, :])
            bqk_sb = constp.tile([J, 2], F32, tag="bqk")
            nc.sync.dma_start(out=bqk_sb, in_=bqk[:, :])
            bv_sb = constp.tile([128, J], F32, tag="bv")
            nc.sync.dma_start(out=bv_sb, in_=bv[0, :].partition_broadcast(128))
            mbig_sb = constp.tile([KB, 896], F32, tag="mbig")
            nc.sync.dma_start(out=mbig_sb, in_=mbig[:, :])
            ones_sb = constp.tile([1, 65], F32R, tag="ones")
            nc.sync.dma_start(out=ones_sb, in_=onesd[:, :])

            qT = persist.tile([128, S], F32R, tag="qT")
            kT = persist.tile([128, S], F32R, tag="kT")
            NKBT = S // KB  # 32
            vsb = persist.tile([128, NKBT, HPC, 66], F32R, tag="vsb")
            # ones column of v_aug (col 64); 1.0 is exact in any rounding
            nc.vector.memset(vsb[:, :, :, 64:65].bitcast(F32), 1.0)

            for _ in range(reps):
                _emit_iteration(nc, tc, xp, ptp, attnp, up, rcp, ypool,
                                scoresp, outtp, mmp, xT, y, wqk_sb, wv_sb,
                                wo_sb, bqk_sb, bv_sb, mbig_sb, ones_sb,
                                qT, kT, vsb)

    _split_waits(nc)
    return nc


def _emit_iteration(nc, tc, xp, ptp, attnp, up, rcp, ypool, scoresp, outtp,
                    mmp, xT, y, wqk_sb, wv_sb, wo_sb, bqk_sb, bv_sb, mbig_sb,
                    ones_sb, qT, kT, vsb):
    x_tiles = {}

    def emit_x_dma(c):
        if c >= NSPAN:
            return
        x_t = xp.tile([128, 8, SPAN], F32R, tag="x")
        src = xT.rearrange("(a p) s -> p a s", p=128)[:, :, c * SPAN:(c + 1) * SPAN]
        # two DMAs so two HW queues run in parallel
        nc.sync.dma_start(out=x_t[:, 0:4, :], in_=src[:, 0:4, :])
        nc.sync.dma_start(out=x_t[:, 4:8, :], in_=src[:, 4:8, :])
        x_tiles[c] = x_t

    def qkv_units(c):
        """6 units: qk j-tile 0, qk j-tile 1, v s-tiles 0..3 of chunk c."""
        x_t = x_tiles[c]
        units = []

        def qk_unit(jt):
            def emit():
                ps = mmp.tile([128, SPAN], F32, tag="mm")
                for a in range(8):
                    nc.tensor.matmul(ps, wqk_sb[:, a, jt * 128:(jt + 1) * 128],
                                     x_t[:, a, :], start=(a == 0), stop=(a == 7))
                dst = (qT if jt == 0 else kT)[:, c * SPAN:(c + 1) * SPAN]
                nc.vector.tensor_scalar_add(out=dst, in0=ps,
                                            scalar1=bqk_sb[:, jt:jt + 1])
            return emit

        def v_unit(t4):
            def emit():
                t = c * 4 + t4
                ps = mmp.tile([128, SPAN], F32, tag="mm")
                for a in range(8):
                    nc.tensor.matmul(ps[:, 0:J], x_t[:, a, t4 * 128:(t4 + 1) * 128],
                                     wv_sb[:, a, :], start=(a == 0), stop=(a == 7))
                nc.vector.tensor_add(
                    out=vsb[:, t, :, 0:64],
                    in0=ps[:, 0:J].rearrange("p (h d) -> p h d", h=HPC),
                    in1=bv_sb.rearrange("p (h d) -> p h d", h=HPC),
                )
            return emit

        units.append(qk_unit(0))
        units.append(qk_unit(1))
        for t4 in range(4):
            units.append(v_unit(t4))
        return units

    def wo_units(c):
        """8 units: [t4 x half] output projection + y DMA for span c."""
        attn = attn_tiles[c]
        units = []

        def wo_unit(t4, half):
            def emit():
                t = c * 4 + t4
                yp = mmp.tile([128, SPAN], F32, tag="mm")
                nc.tensor.matmul(yp, attn[:, t4 * 128:(t4 + 1) * 128],
                                 wo_sb[:, half * SPAN:(half + 1) * SPAN],
                                 start=True, stop=True)
                ysb = ypool.tile([128, SPAN], F32, tag="ysb")
                nc.vector.tensor_copy(out=ysb, in_=yp)
                nc.sync.dma_start(
                    out=y[t * 128:(t + 1) * 128, half * SPAN:(half + 1) * SPAN],
                    in_=ysb)
            return emit

        for t4 in range(4):
            for half in range(2):
                units.append(wo_unit(t4, half))
        return units

    attn_tiles = {}

    # ---- prologue: chunk 0 qkv + x DMAs ----
    emit_x_dma(0)
    emit_x_dma(1)
    for u in qkv_units(0):
        u()

    # ---- spans ----
    for c in range(NSPAN):
        nkb = 4 * (c + 1)
        ngrp = nkb // 2

        emit_x_dma(c + 2)
        units = []
        if c + 1 < NSPAN:
            qu = qkv_units(c + 1)
        else:
            qu = []
        wu = wo_units(c - 1) if c >= 1 else []
        # interleave: qk units early, wo and v spread
        mix = []
        qi = wi = 0
        while qi < len(qu) or wi < len(wu):
            if qi < len(qu):
                mix.append(qu[qi]); qi += 1
            if wi < len(wu):
                mix.append(wu[wi]); wi += 1
            if wi < len(wu):
                mix.append(wu[wi]); wi += 1
        units = mix

        outT = [outtp.tile([65, SPAN], F32, tag="outT", name=f"outT{_h}") for _h in range(HPC)]
        prev = None  # (g, [pt_h0, pt_h1])
        udone = 0

        def flush_pv(g, pts):
            for h in range(HPC):
                for i in range(2):
                    kb = 2 * g + i
                    nc.tensor.matmul(outT[h], vsb[:, kb, h, 0:65],
                                     pts[h][:, i * SPAN:(i + 1) * SPAN],
                                     start=(kb == 0), stop=(kb == nkb - 1))

        for g in range(ngrp):
            pts = []
            qps_l = []
            for h in range(HPC):
                qps = scoresp.tile([128, 2 * SPAN], F32, tag="sc")
                for i in range(2):
                    kb = 2 * g + i
                    nc.tensor.matmul(
                        qps[:, i * SPAN:(i + 1) * SPAN],
                        kT[h * HD:(h + 1) * HD, kb * KB:(kb + 1) * KB],
                        qT[h * HD:(h + 1) * HD, c * SPAN:(c + 1) * SPAN],
                        start=True, stop=True)
                qps_l.append(qps)
            for h in range(HPC):
                qps = qps_l[h]
                for i in range(2):
                    kb = 2 * g + i
                    off = kb * KB - c * SPAN
                    if off >= 0:  # diagonal block: additive causal mask
                        nc.vector.tensor_add(
                            out=qps[:, i * SPAN:(i + 1) * SPAN],
                            in0=qps[:, i * SPAN:(i + 1) * SPAN],
                            in1=mbig_sb[:, 384 - off:896 - off])
                pt = ptp.tile([128, 2 * SPAN], F32R, tag="pt")
                nc.scalar.activation(out=pt, in_=qps,
                                     func=mybir.ActivationFunctionType.Exp,
                                     scale=float(1.0 / np.sqrt(HD)))
                pts.append(pt)
            if prev is not None:
                flush_pv(*prev)
            prev = (g, pts)
            # sprinkle scheduled units
            target = ((g + 1) * len(units)) // ngrp
            while udone < target:
                units[udone]()
                udone += 1
        while udone < len(units):
            units[udone]()
            udone += 1
        flush_pv(*prev)

        # ---- normalize span c ----
        attn = attnp.tile([128, SPAN], F32R, tag="attn")
        for h in range(HPC):
            u = up.tile([65, SPAN], F32, tag="u")
            nc.vector.tensor_copy(out=u, in_=outT[h])
            rc = rcp.tile([1, SPAN], F32R, tag="rc")
            nc.vector.reciprocal(out=rc, in_=u[64:65, :])
            bc = mmp.tile([128, SPAN], F32, tag="mm")
            nc.tensor.matmul(bc[0:64, :], ones_sb[0:1, 0:64], rc,
                             start=True, stop=True)
            nc.vector.tensor_mul(out=attn[h * HD:(h + 1) * HD, :],
                                 in0=u[0:64, :], in1=bc[0:64, :])
        attn_tiles[c] = attn

    # ---- epilogue: wo for last span ----
    for u in wo_units(NSPAN - 1):
        u()


# ---------------------------------------------------------------- host side
def _prep_core_inputs(r, xTf, Wqkv_w, Wqkv_b, Wo_w):
    g0, g1 = HPC * r, HPC * r + 1
    Wq, Wk, Wv = Wqkv_w[0:C], Wqkv_w[C:2 * C], Wqkv_w[2 * C:3 * C]
    bq, bk, bvv = Wqkv_b[0:C], Wqkv_b[C:2 * C], Wqkv_b[2 * C:3 * C]
    rows0 = slice(HD * g0, HD * g0 + HD)
    rows1 = slice(HD * g1, HD * g1 + HD)
    wqk = np.concatenate(
        [Wq[rows0].T, Wq[rows1].T, Wk[rows0].T, Wk[rows1].T], axis=1)
    wv = np.concatenate([Wv[rows0].T, Wv[rows1].T], axis=1)
    bqk = np.stack(
        [np.concatenate([bq[rows0], bq[rows1]]),
         np.concatenate([bk[rows0], bk[rows1]])], axis=1)
    bv = np.concatenate([bvv[rows0], bvv[rows1]])[None, :]
    wo = np.concatenate([Wo_w[:, rows0], Wo_w[:, rows1]], axis=1).T
    return {
        "xT": np.ascontiguousarray(xTf),
        "wqk": np.ascontiguousarray(wqk, np.float32),
        "wv": np.ascontiguousarray(wv, np.float32),
        "wo": np.ascontiguousarray(wo, np.float32),
        "bqk": np.ascontiguousarray(bqk, np.float32),
        "bv": np.ascontiguousarray(bv, np.float32),
        "mbig": _mbig(),
        "onesd": np.ones((1, 65), np.float32),
    }


def _mbig():
    m = np.full((KB, 896), NEG, np.float32)
    i = np.arange(KB)[:, None]
    cidx = np.arange(896)[None, :]
    m[cidx >= i + 384] = 0.0
    return m


def make_in_maps(x, Wqkv_w, Wqkv_b, Wo_w):
    xTf = np.ascontiguousarray(np.asarray(x, np.float32)[0].T)
    return [_prep_core_inputs(r, xTf, np.asarray(Wqkv_w, np.float32),
                              np.asarray(Wqkv_b, np.float32),
                              np.asarray(Wo_w, np.float32))
            for r in range(NCORES)]


_NC_CACHE = {}


def kernel(x, mask, Wqkv_w, Wqkv_b, Wo_w, Wo_b):
    from concourse.bass_utils import run_bass_kernel_spmd
    # The padding mask is all-False for this problem (spec fill=zeros);
    # causal masking is handled on-device.
    if 1 not in _NC_CACHE:
        _NC_CACHE[1] = build_nc(1)
    nc = _NC_CACHE[1]
    in_maps = make_in_maps(x, Wqkv_w, Wqkv_b, Wo_w)
    res = run_bass_kernel_spmd(nc, in_maps, core_ids=list(range(NCORES)))
    out = np.zeros((S, C), np.float64)
    for r in range(NCORES):
        out += res.results[r]["y"].astype(np.float64)
    out += np.asarray(Wo_b, np.float32).astype(np.float64)
    return out.astype(np.float32)[None, :, :]


# revision 18
# speedup vs baseline: 426.0848x; 426.0848x over previous
"""Causal self-attention (B=1, S=4096, C=1024, NH=16) on 8 Trainium2
NeuronCores.

Sharding: heads 2-per-core (tensor parallel). Wqkv column-sharded,
Wo row-sharded; each core computes a full-shape partial of the output
projection and the host sums the 8 partials (+ Wo bias).

Per-core dataflow (all matmuls in float32r — fp32 storage, tf32-class
matmul precision at full PE rate):
  xT (C on partitions, host-pretransposed) -> qT/kT [128=2*64hd, S] and
  v [S, hd+ones] via the QKV projection; flash-style attention with
  k-major score tiles scoresT[sk,sq] so softmax denominators ride the
  PV matmul as an appended ones column of V; exp straight from PSUM on
  ScalarE; causal masking via an additive sliding-window mask on the
  diagonal k-blocks; out^T = v_aug.T @ exp(scoresT) accumulates in PSUM
  [65, span]; normalization via reciprocal + K=1 broadcast matmul; the
  output projection consumes attn^T directly and partial y rows DMA
  from PSUM to DRAM.
"""
import sys

sys.path.insert(0, "/opt/trn_rl_repo")

import numpy as np

import concourse.bass as bass
import concourse.mybir as mybir
from concourse import tile

F32 = mybir.dt.float32
F32R = mybir.dt.float32r

S = 4096
C = 1024
NH = 16
HD = 64
NCORES = 8
HPC = NH // NCORES          # heads per core = 2
J = HPC * HD                # 128 qkv rows per section per core
SPAN = 512                  # q-span / s-chunk
NSPAN = S // SPAN           # 8
KB = 128                    # k-block
NEG = -1.0e9


# ---------------------------------------------------------------- fixups
_WAIT_LIMITS = {}
_WAIT_DEFAULT = 1


def _split_waits(nc, max_waits=None):
    """This container's walrus rejects >1 sync-wait on some instruction
    structs (CTRL drains, f32r self-loading matmuls); hoist excess waits onto
    single-wait EventSemaphore carriers inserted just before the instruction
    (same engine)."""
    wid = 0
    for f in nc.m.functions:
        for bb in f.blocks:
            insts = bb.instructions
            i = 0
            while i < len(insts):
                ins = insts[i]
                si = getattr(ins, "sync_info", None)
                max_waits = _WAIT_LIMITS.get(type(ins).__name__, _WAIT_DEFAULT)
                if si is not None and len(si.on_wait) > max_waits:
                    waits = list(si.on_wait)
                    si.on_wait = waits[:max_waits]
                    for w in waits[max_waits:]:
                        wid += 1
                        insts.insert(i, mybir.InstEventSemaphore(
                            name=f"WSPLIT-{wid}",
                            engine=ins.engine,
                            ins=[], outs=[],
                            sync_info=mybir.SyncInfo(on_wait=[w], on_update=[]),
                        ))
                        i += 1
                i += 1


# ---------------------------------------------------------------- program
def build_nc(reps: int = 1) -> bass.Bass:
    nc = bass.Bass()
    xT = nc.dram_tensor("xT", [C, S], F32R, kind="ExternalInput")
    wqkv = nc.dram_tensor("wqkv", [C, 3 * J], F32R, kind="ExternalInput")
    wo = nc.dram_tensor("wo", [J, C], F32R, kind="ExternalInput")
    bqkv = nc.dram_tensor("bqkv", [J, 3], F32, kind="ExternalInput")
    eye = nc.dram_tensor("eye", [128, 128], F32R, kind="ExternalInput")
    mbig = nc.dram_tensor("mbig", [KB, 896], F32, kind="ExternalInput")
    onesd = nc.dram_tensor("onesd", [1, 65], F32R, kind="ExternalInput")
    y = nc.dram_tensor("y", [S, C], F32, kind="ExternalOutput")

    with tile.TileContext(nc) as tc:
        with (
            nc.allow_low_precision(reason="f32r is full-rate on PE; rounding error is acceptable here"),
            tc.tile_pool(name="const", bufs=1) as constp,
            tc.tile_pool(name="persist", bufs=1) as persist,
            tc.tile_pool(name="xp", bufs=2) as xp,
            tc.tile_pool(name="vtp", bufs=3) as vtp,
            tc.tile_pool(name="ptp", bufs=9) as ptp,
            tc.tile_pool(name="attnp", bufs=3) as attnp,
            tc.tile_pool(name="up", bufs=3) as up,
            tc.tile_pool(name="yp", bufs=6) as ypool,
            tc.tile_pool(name="rcp", bufs=3) as rcp,
            tc.tile_pool(name="scores", bufs=2, space="PSUM") as scoresp,
            tc.tile_pool(name="outtp", bufs=2, space="PSUM") as outtp,
            tc.tile_pool(name="mmp", bufs=2, space="PSUM") as mmp,
        ):
            # ---- constants (wqkv emitted interleaved with the first x
            # chunk inside _emit_iteration via late_consts) ----
            wqkv_sb = constp.tile([128, 8, 3 * J], F32R, tag="wqkv")
            wo_sb = constp.tile([J, C], F32R, tag="wo")
            bqkv_sb = constp.tile([J, 3], F32, tag="bqkv")
            eye_sb = constp.tile([128, 128], F32R, tag="eye")
            mbig_sb = constp.tile([KB, 896], F32, tag="mbig")
            ones_sb = constp.tile([1, 65], F32R, tag="ones")
            nc.sync.dma_start(out=bqkv_sb, in_=bqkv[:, :])

            def late_consts():
                nc.sync.dma_start(out=eye_sb, in_=eye[:, :])
                nc.sync.dma_start(out=mbig_sb, in_=mbig[:, :])
                nc.sync.dma_start(out=ones_sb, in_=onesd[:, :])
                nc.sync.dma_start(out=wo_sb, in_=wo[:, :])

            qT = persist.tile([128, S], F32R, tag="qT")
            kT = persist.tile([128, S], F32R, tag="kT")
            NKBT = S // KB  # 32
            vsb = persist.tile([128, NKBT, HPC, 66], F32R, tag="vsb")
            # ones column of v_aug (col 64); 1.0 is exact in any rounding
            nc.vector.memset(vsb[:, :, :, 64:65].bitcast(F32), 1.0)

            for _ in range(reps):
                _emit_iteration(nc, tc, xp, vtp, ptp, attnp, up, rcp, ypool,
                                scoresp, outtp, mmp, xT, y, wqkv_sb,
                                wo_sb, bqkv_sb, mbig_sb, ones_sb, eye_sb,
                                qT, kT, vsb, wqkv, late_consts)

    _split_waits(nc)
    return nc


def _emit_iteration(nc, tc, xp, vtp, ptp, attnp, up, rcp, ypool, scoresp,
                    outtp, mmp, xT, y, wqkv_sb, wo_sb, bqkv_sb, mbig_sb,
                    ones_sb, eye_sb, qT, kT, vsb, wqkv=None, late_consts=None):
    x_tiles = {}

    def emit_x_dma(c):
        if c >= NSPAN:
            return
        x_t = xp.tile([128, 8, SPAN], F32R, tag="x")
        src = xT.rearrange("(a p) s -> p a s", p=128)[:, :, c * SPAN:(c + 1) * SPAN]
        if c == 0:
            # per-a pieces so the first projection matmul starts ASAP
            for _a in range(8):
                nc.sync.dma_start(out=x_t[:, _a, :], in_=src[:, _a, :])
        else:
            nc.sync.dma_start(out=x_t[:, 0:4, :], in_=src[:, 0:4, :])
            nc.sync.dma_start(out=x_t[:, 4:8, :], in_=src[:, 4:8, :])
        x_tiles[c] = x_t

    def qkv_units(c):
        """7 units: q/k/vT projections (N=512) + 4 v transposes of chunk c."""
        x_t = x_tiles[c]
        vT_c = vtp.tile([128, SPAN], F32R, tag="vt", name=f"vt{c}")
        units = []

        def proj_unit(jt):
            def emit():
                ps = mmp.tile([128, SPAN], F32, tag="mm")
                for a in range(8):
                    nc.tensor.matmul(ps, wqkv_sb[:, a, jt * 128:(jt + 1) * 128],
                                     x_t[:, a, :], start=(a == 0), stop=(a == 7))
                dst = (qT, kT)[jt][:, c * SPAN:(c + 1) * SPAN] if jt < 2 else vT_c
                nc.vector.tensor_scalar_add(out=dst, in0=ps,
                                            scalar1=bqkv_sb[:, jt:jt + 1])
            return emit

        def vtr_unit(t4):
            def emit():
                t = c * 4 + t4
                ps = mmp.tile([128, SPAN], F32, tag="mm")
                nc.tensor.transpose(ps[:, 0:128].bitcast(F32R),
                                    vT_c[:, t4 * 128:(t4 + 1) * 128], eye_sb)
                nc.vector.tensor_copy(
                    out=vsb[:, t, :, 0:64],
                    in_=ps[:, 0:J].rearrange("p (h d) -> p h d", h=HPC))
            return emit

        for jt in range(3):
            units.append(proj_unit(jt))
        for t4 in range(4):
            units.append(vtr_unit(t4))
        return units

    def wo_units(c):
        """8 units: [t4 x half] output projection + y DMA for span c."""
        units = []

        def wo_unit(t4, half):
            def emit():
                attn = attn_tiles[c]
                t = c * 4 + t4
                yp = mmp.tile([128, SPAN], F32, tag="mm")
                nc.tensor.matmul(yp, attn[:, t4 * 128:(t4 + 1) * 128],
                                 wo_sb[:, half * SPAN:(half + 1) * SPAN],
                                 start=True, stop=True)
                ysb = ypool.tile([128, SPAN], F32, tag="ysb")
                nc.vector.tensor_copy(out=ysb, in_=yp)
                nc.sync.dma_start(
                    out=y[t * 128:(t + 1) * 128, half * SPAN:(half + 1) * SPAN],
                    in_=ysb)
            return emit

        for t4 in range(4):
            for half in range(2):
                units.append(wo_unit(t4, half))
        return units

    attn_tiles = {}
    norm_thunks = []
    wo_fifo = []

    def make_normalize(c, outT):
        def norm():
            attn = attnp.tile([128, SPAN], F32R, tag="attn")
            rcs, us, bcs = [], [], []
            for h in range(HPC):
                rc = rcp.tile([1, SPAN], F32R, tag="rc", name=f"rc{h}")
                nc.vector.reciprocal(out=rc, in_=outT[h][64:65, :])
                rcs.append(rc)
            for h in range(HPC):
                bc = mmp.tile([64, SPAN], F32, tag="mm", name=f"bc{h}")
                nc.tensor.matmul(bc, ones_sb[0:1, 0:64], rcs[h],
                                 start=True, stop=True)
                bcs.append(bc)
                u = up.tile([65, SPAN], F32, tag="u", name=f"u{h}")
                nc.vector.tensor_copy(out=u, in_=outT[h])
                us.append(u)
            for h in range(HPC):
                nc.vector.tensor_mul(out=attn[h * HD:(h + 1) * HD, :],
                                     in0=us[h][0:64, :], in1=bcs[h])
            attn_tiles[c] = attn
            wo_fifo.extend(wo_units(c))
        return norm

    # ---- prologue: chunk 0 qkv + x DMAs ----
    if wqkv is not None:
        # interleave first x chunk pieces with wqkv pieces so the first
        # projection matmul's operands arrive first on the DMA engines
        x_t = xp.tile([128, 8, SPAN], F32R, tag="x", name="x0")
        xsrc = xT.rearrange("(a p) s -> p a s", p=128)[:, :, 0:SPAN]
        wsrc = wqkv.rearrange("(a p) j -> p a j", p=128)
        for _a in range(8):
            nc.sync.dma_start(out=x_t[:, _a, :], in_=xsrc[:, _a, :])
            nc.sync.dma_start(out=wqkv_sb[:, _a, :], in_=wsrc[:, _a, :])
        x_tiles[0] = x_t
        if late_consts is not None:
            late_consts()
    else:
        emit_x_dma(0)
    emit_x_dma(1)
    for u in qkv_units(0):
        u()

    # ---- spans ----
    for c in range(NSPAN):
        nkb = 4 * (c + 1)
        ngrp = nkb // 2

        emit_x_dma(c + 2)
        units = qkv_units(c + 1) if c + 1 < NSPAN else []

        outT = [outtp.tile([65, SPAN], F32, tag="outT", name=f"outT{_h}") for _h in range(HPC)]
        pending = []  # [(g, [pt_h0, pt_h1])], PV lags 2 quads behind QK/exp
        udone = 0
        outT_c = outT  # capture for the deferred normalize

        def flush_pv(g, pts):
            for h in range(HPC):
                for i in range(2):
                    kb = 2 * g + i
                    off = max(0, kb * KB - c * SPAN)
                    nc.tensor.matmul(outT[h][:, off:SPAN],
                                     vsb[:, kb, h, 0:65],
                                     pts[h][:, i * SPAN + off:(i + 1) * SPAN],
                                     start=(kb == 0), stop=(kb == nkb - 1))

        for g in range(ngrp):
            pts = []
            qps_l = []
            for h in range(HPC):
                qps = scoresp.tile([128, 2 * SPAN], F32, tag="sc")
                for i in range(2):
                    kb = 2 * g + i
                    nc.tensor.matmul(
                        qps[:, i * SPAN:(i + 1) * SPAN],
                        kT[h * HD:(h + 1) * HD, kb * KB:(kb + 1) * KB],
                        qT[h * HD:(h + 1) * HD, c * SPAN:(c + 1) * SPAN],
                        start=True, stop=True)
                qps_l.append(qps)
            for h in range(HPC):
                qps = qps_l[h]
                for i in range(2):
                    kb = 2 * g + i
                    off = kb * KB - c * SPAN
                    if off >= 0:  # diagonal block: triangle mask on its 128 cols
                        nc.vector.tensor_add(
                            out=qps[:, i * SPAN + off:i * SPAN + off + KB],
                            in0=qps[:, i * SPAN + off:i * SPAN + off + KB],
                            in1=mbig_sb[:, 384:512])
                pt = ptp.tile([128, 2 * SPAN], F32R, tag="pt")
                nc.scalar.activation(out=pt, in_=qps,
                                     func=mybir.ActivationFunctionType.Exp,
                                     scale=float(1.0 / np.sqrt(HD)))
                pts.append(pt)
            if g == 0 and norm_thunks:
                # previous span's normalize, overlapped into this span's pipe
                norm_thunks.pop(0)()
            pending.append((g, pts))
            if len(pending) > 2:
                flush_pv(*pending.pop(0))
            # sprinkle qkv units (reserve a share for the span tail) and
            # drain roughly one deferred wo unit per quad
            target = ((g + 1) * len(units)) // (ngrp + 2)
            while udone < target:
                units[udone]()
                udone += 1
            if wo_fifo:
                wo_fifo.pop(0)()
            if len(wo_fifo) > 10:
                wo_fifo.pop(0)()
        for k, item in enumerate(pending):
            flush_pv(*item)
            target = ((ngrp + 1 + k) * len(units)) // (ngrp + 2)
            while udone < target:
                units[udone]()
                udone += 1
            if wo_fifo:
                wo_fifo.pop(0)()

        while udone < len(units):
            units[udone]()
            udone += 1
        norm_thunks.append(make_normalize(c, outT_c))

    # ---- epilogue: last normalize + remaining wo units ----
    for t in norm_thunks:
        t()
    norm_thunks.clear()
    while wo_fifo:
        wo_fifo.pop(0)()




# ---------------------------------------------------------------- host side
def _prep_core_inputs(r, xTf, Wqkv_w, Wqkv_b, Wo_w):
    g0, g1 = HPC * r, HPC * r + 1
    Wq, Wk, Wv = Wqkv_w[0:C], Wqkv_w[C:2 * C], Wqkv_w[2 * C:3 * C]
    bq, bk, bvv = Wqkv_b[0:C], Wqkv_b[C:2 * C], Wqkv_b[2 * C:3 * C]
    rows0 = slice(HD * g0, HD * g0 + HD)
    rows1 = slice(HD * g1, HD * g1 + HD)
    wqkv = np.concatenate(
        [Wq[rows0].T, Wq[rows1].T, Wk[rows0].T, Wk[rows1].T,
         Wv[rows0].T, Wv[rows1].T], axis=1)
    bqkv = np.stack(
        [np.concatenate([bq[rows0], bq[rows1]]),
         np.concatenate([bk[rows0], bk[rows1]]),
         np.concatenate([bvv[rows0], bvv[rows1]])], axis=1)
    wo = np.concatenate([Wo_w[:, rows0], Wo_w[:, rows1]], axis=1).T
    return {
        "xT": np.ascontiguousarray(xTf),
        "wqkv": np.ascontiguousarray(wqkv, np.float32),
        "wo": np.ascontiguousarray(wo, np.float32),
        "bqkv": np.ascontiguousarray(bqkv, np.float32),
        "mbig": _mbig(),
        "onesd": np.ones((1, 65), np.float32),
        "eye": np.eye(128, dtype=np.float32),
    }


def _mbig():
    m = np.full((KB, 896), NEG, np.float32)
    i = np.arange(KB)[:, None]
    cidx = np.arange(896)[None, :]
    m[cidx >= i + 384] = 0.0
    return m


def make_in_maps(x, Wqkv_w, Wqkv_b, Wo_w):
    xTf = np.ascontiguousarray(np.asarray(x, np.float32)[0].T)
    return [_prep_core_inputs(r, xTf, np.asarray(Wqkv_w, np.float32),
                              np.asarray(Wqkv_b, np.float32),
                              np.asarray(Wo_w, np.float32))
            for r in range(NCORES)]


_NC_CACHE = {}


def kernel(x, mask, Wqkv_w, Wqkv_b, Wo_w, Wo_b):
    from concourse.bass_utils import run_bass_kernel_spmd
    # The padding mask is all-False for this problem (spec fill=zeros);
    # causal masking is handled on-device.
    if 1 not in _NC_CACHE:
        _NC_CACHE[1] = build_nc(1)
    nc = _NC_CACHE[1]
    in_maps = make_in_maps(x, Wqkv_w, Wqkv_b, Wo_w)
    res = run_bass_kernel_spmd(nc, in_maps, core_ids=list(range(NCORES)))
    out = np.zeros((S, C), np.float64)
    for r in range(NCORES):
        out += res.results[r]["y"].astype(np.float64)
    out += np.asarray(Wo_b, np.float32).astype(np.float64)
    return out.astype(np.float32)[None, :, :]


# revision 20
# speedup vs baseline: 494.9698x; 1.1617x over previous
"""Causal self-attention (B=1, S=4096, C=1024, NH=16) on 8 Trainium2
NeuronCores.

Sharding: heads 2-per-core (tensor parallel). Wqkv column-sharded,
Wo row-sharded; each core computes a full-shape partial of the output
projection and the host sums the 8 partials (+ Wo bias).

Per-core dataflow (all matmuls in float32r — fp32 storage, tf32-class
matmul precision at full PE rate):
  xT (C on partitions, host-pretransposed) -> qT/kT [128=2*64hd, S] and
  v [S, hd+ones] via the QKV projection; flash-style attention with
  k-major score tiles scoresT[sk,sq] so softmax denominators ride the
  PV matmul as an appended ones column of V; exp straight from PSUM on
  ScalarE; causal masking via an additive sliding-window mask on the
  diagonal k-blocks; out^T = v_aug.T @ exp(scoresT) accumulates in PSUM
  [65, span]; normalization via reciprocal + K=1 broadcast matmul; the
  output projection consumes attn^T directly and partial y rows DMA
  from PSUM to DRAM.
"""
import sys

sys.path.insert(0, "/opt/trn_rl_repo")

import numpy as np

import concourse.bass as bass
import concourse.mybir as mybir
from concourse import tile

F32 = mybir.dt.float32
F32R = mybir.dt.float32r

S = 4096
C = 1024
NH = 16
HD = 64
NCORES = 8
HPC = NH // NCORES          # heads per core = 2
J = HPC * HD                # 128 qkv rows per section per core
SPAN = 512                  # q-span / s-chunk
NSPAN = S // SPAN           # 8
KB = 128                    # k-block
NEG = -1.0e9


# ---------------------------------------------------------------- fixups
_WAIT_LIMITS = {}
_WAIT_DEFAULT = 1


def _split_waits(nc, max_waits=None):
    """This container's walrus rejects >1 sync-wait on some instruction
    structs (CTRL drains, f32r self-loading matmuls); hoist excess waits onto
    single-wait EventSemaphore carriers inserted just before the instruction
    (same engine)."""
    wid = 0
    for f in nc.m.functions:
        for bb in f.blocks:
            insts = bb.instructions
            i = 0
            while i < len(insts):
                ins = insts[i]
                si = getattr(ins, "sync_info", None)
                max_waits = _WAIT_LIMITS.get(type(ins).__name__, _WAIT_DEFAULT)
                if si is not None and len(si.on_wait) > max_waits:
                    waits = list(si.on_wait)
                    si.on_wait = waits[:max_waits]
                    for w in waits[max_waits:]:
                        wid += 1
                        insts.insert(i, mybir.InstEventSemaphore(
                            name=f"WSPLIT-{wid}",
                            engine=ins.engine,
                            ins=[], outs=[],
                            sync_info=mybir.SyncInfo(on_wait=[w], on_update=[]),
                        ))
                        i += 1
                i += 1


# ---------------------------------------------------------------- program
def build_nc(reps: int = 1) -> bass.Bass:
    nc = bass.Bass()
    xT = nc.dram_tensor("xT", [C, S], F32R, kind="ExternalInput")
    wqkv = nc.dram_tensor("wqkv", [C, 3 * J], F32R, kind="ExternalInput")
    wo = nc.dram_tensor("wo", [J, C], F32R, kind="ExternalInput")
    bqkv = nc.dram_tensor("bqkv", [J, 3], F32, kind="ExternalInput")
    eye = nc.dram_tensor("eye", [128, 128], F32R, kind="ExternalInput")
    mbig = nc.dram_tensor("mbig", [KB, 896], F32, kind="ExternalInput")
    onesd = nc.dram_tensor("onesd", [1, 65], F32R, kind="ExternalInput")
    y = nc.dram_tensor("y", [S, C], F32, kind="ExternalOutput")

    with tile.TileContext(nc) as tc:
        with (
            nc.allow_low_precision(reason="f32r is full-rate on PE; rounding error is acceptable here"),
            tc.tile_pool(name="const", bufs=1) as constp,
            tc.tile_pool(name="persist", bufs=1) as persist,
            tc.tile_pool(name="xp", bufs=2) as xp,
            tc.tile_pool(name="vtp", bufs=3) as vtp,
            tc.tile_pool(name="ptp", bufs=9) as ptp,
            tc.tile_pool(name="attnp", bufs=3) as attnp,
            tc.tile_pool(name="up", bufs=3) as up,
            tc.tile_pool(name="yp", bufs=6) as ypool,
            tc.tile_pool(name="rcp", bufs=3) as rcp,
            tc.tile_pool(name="scores", bufs=2, space="PSUM") as scoresp,
            tc.tile_pool(name="outtp", bufs=2, space="PSUM") as outtp,
            tc.tile_pool(name="mmp", bufs=2, space="PSUM") as mmp,
        ):
            # ---- constants (wqkv emitted interleaved with the first x
            # chunk inside _emit_iteration via late_consts) ----
            wqkv_sb = constp.tile([128, 8, 3 * J], F32R, tag="wqkv")
            wo_sb = constp.tile([J, C], F32R, tag="wo")
            bqkv_sb = constp.tile([J, 3], F32, tag="bqkv")
            eye_sb = constp.tile([128, 128], F32R, tag="eye")
            mbig_sb = constp.tile([KB, 896], F32, tag="mbig")
            ones_sb = constp.tile([1, 65], F32R, tag="ones")
            nc.sync.dma_start(out=bqkv_sb, in_=bqkv[:, :])

            def late_consts():
                nc.sync.dma_start(out=eye_sb, in_=eye[:, :])
                nc.sync.dma_start(out=mbig_sb, in_=mbig[:, :])
                nc.sync.dma_start(out=ones_sb, in_=onesd[:, :])
                nc.sync.dma_start(out=wo_sb, in_=wo[:, :])

            qT = persist.tile([128, S], F32R, tag="qT")
            kT = persist.tile([128, S], F32R, tag="kT")
            NKBT = S // KB  # 32
            vsb = persist.tile([128, NKBT, HPC, 66], F32R, tag="vsb")
            # ones column of v_aug (col 64); 1.0 is exact in any rounding
            nc.vector.memset(vsb[:, :, :, 64:65].bitcast(F32), 1.0)

            for _ in range(reps):
                _emit_iteration(nc, tc, xp, vtp, ptp, attnp, up, rcp, ypool,
                                scoresp, outtp, mmp, xT, y, wqkv_sb,
                                wo_sb, bqkv_sb, mbig_sb, ones_sb, eye_sb,
                                qT, kT, vsb, wqkv, late_consts)

    _split_waits(nc)
    return nc


def _emit_iteration(nc, tc, xp, vtp, ptp, attnp, up, rcp, ypool, scoresp,
                    outtp, mmp, xT, y, wqkv_sb, wo_sb, bqkv_sb, mbig_sb,
                    ones_sb, eye_sb, qT, kT, vsb, wqkv=None, late_consts=None):
    x_tiles = {}

    def emit_x_dma(c):
        if c >= NSPAN:
            return
        x_t = xp.tile([128, 8, SPAN], F32R, tag="x")
        src = xT.rearrange("(a p) s -> p a s", p=128)[:, :, c * SPAN:(c + 1) * SPAN]
        if c == 0:
            # per-a pieces so the first projection matmul starts ASAP
            for _a in range(8):
                nc.sync.dma_start(out=x_t[:, _a, :], in_=src[:, _a, :])
        else:
            nc.sync.dma_start(out=x_t[:, 0:4, :], in_=src[:, 0:4, :])
            nc.sync.dma_start(out=x_t[:, 4:8, :], in_=src[:, 4:8, :])
        x_tiles[c] = x_t

    def qkv_units(c):
        """7 units: q/k/vT projections (N=512) + 4 v transposes of chunk c."""
        x_t = x_tiles[c]
        vT_c = vtp.tile([128, SPAN], F32R, tag="vt", name=f"vt{c}")
        units = []

        def proj_unit(jt):
            def emit():
                ps = mmp.tile([128, SPAN], F32, tag="mm")
                if c == 0 and jt == 0:
                    # N=256 halves: the very first matmul only needs the
                    # first half of the first x piece to have landed
                    for half in range(2):
                        sl = slice(half * 256, (half + 1) * 256)
                        for a in range(8):
                            nc.tensor.matmul(
                                ps[:, sl], wqkv_sb[:, a, jt * 128:(jt + 1) * 128],
                                x_t[:, a, sl], start=(a == 0), stop=(a == 7))
                else:
                    for a in range(8):
                        nc.tensor.matmul(ps, wqkv_sb[:, a, jt * 128:(jt + 1) * 128],
                                         x_t[:, a, :], start=(a == 0), stop=(a == 7))
                dst = (qT, kT)[jt][:, c * SPAN:(c + 1) * SPAN] if jt < 2 else vT_c
                nc.vector.tensor_scalar_add(out=dst, in0=ps,
                                            scalar1=bqkv_sb[:, jt:jt + 1])
            return emit

        def vtr_unit(t4):
            def emit():
                t = c * 4 + t4
                ps = mmp.tile([128, SPAN], F32, tag="mm")
                nc.tensor.transpose(ps[:, 0:128].bitcast(F32R),
                                    vT_c[:, t4 * 128:(t4 + 1) * 128], eye_sb)
                nc.vector.tensor_copy(
                    out=vsb[:, t, :, 0:64],
                    in_=ps[:, 0:J].rearrange("p (h d) -> p h d", h=HPC))
            return emit

        for jt in range(3):
            units.append(proj_unit(jt))
        for t4 in range(4):
            units.append(vtr_unit(t4))
        return units

    def wo_units(c):
        """8 units: [t4 x half] output projection + y DMA for span c."""
        units = []

        def wo_unit(t4, half):
            def emit():
                attn = attn_tiles[c]
                t = c * 4 + t4
                yp = mmp.tile([128, SPAN], F32, tag="mm")
                nc.tensor.matmul(yp, attn[:, t4 * 128:(t4 + 1) * 128],
                                 wo_sb[:, half * SPAN:(half + 1) * SPAN],
                                 start=True, stop=True)
                ysb = ypool.tile([128, SPAN], F32, tag="ysb")
                nc.vector.tensor_copy(out=ysb, in_=yp)
                nc.sync.dma_start(
                    out=y[t * 128:(t + 1) * 128, half * SPAN:(half + 1) * SPAN],
                    in_=ysb)
            return emit

        for t4 in range(4):
            for half in range(2):
                units.append(wo_unit(t4, half))
        return units

    attn_tiles = {}
    norm_thunks = []
    wo_fifo = []

    def make_normalize(c, outT):
        def norm():
            attn = attnp.tile([128, SPAN], F32R, tag="attn")
            rcs, us, bcs = [], [], []
            for h in range(HPC):
                rc = rcp.tile([1, SPAN], F32R, tag="rc", name=f"rc{h}")
                nc.vector.reciprocal(out=rc, in_=outT[h][64:65, :])
                rcs.append(rc)
            for h in range(HPC):
                bc = mmp.tile([64, SPAN], F32, tag="mm", name=f"bc{h}")
                nc.tensor.matmul(bc, ones_sb[0:1, 0:64], rcs[h],
                                 start=True, stop=True)
                bcs.append(bc)
                u = up.tile([65, SPAN], F32, tag="u", name=f"u{h}")
                nc.vector.tensor_copy(out=u, in_=outT[h])
                us.append(u)
            for h in range(HPC):
                nc.vector.tensor_mul(out=attn[h * HD:(h + 1) * HD, :],
                                     in0=us[h][0:64, :], in1=bcs[h])
            attn_tiles[c] = attn
            wo_fifo.extend(wo_units(c))
        return norm

    # ---- prologue: chunk 0 qkv + x DMAs ----
    if wqkv is not None:
        # interleave first x chunk pieces with wqkv pieces so the first
        # projection matmul's operands arrive first on the DMA engines
        x_t = xp.tile([128, 8, SPAN], F32R, tag="x", name="x0")
        xsrc = xT.rearrange("(a p) s -> p a s", p=128)[:, :, 0:SPAN]
        wsrc = wqkv.rearrange("(a p) j -> p a j", p=128)
        for _a in range(8):
            nc.sync.dma_start(out=wqkv_sb[:, _a, :], in_=wsrc[:, _a, :])
            if _a == 0:
                nc.sync.dma_start(out=x_t[:, 0, 0:256], in_=xsrc[:, 0, 0:256])
                nc.sync.dma_start(out=x_t[:, 0, 256:512], in_=xsrc[:, 0, 256:512])
            else:
                nc.sync.dma_start(out=x_t[:, _a, :], in_=xsrc[:, _a, :])
        x_tiles[0] = x_t
        if late_consts is not None:
            late_consts()
    else:
        emit_x_dma(0)
    emit_x_dma(1)
    for u in qkv_units(0):
        u()

    # ---- spans ----
    for c in range(NSPAN):
        nkb = 4 * (c + 1)
        ngrp = nkb // 2

        emit_x_dma(c + 2)
        units = qkv_units(c + 1) if c + 1 < NSPAN else []

        outT = [outtp.tile([65, SPAN], F32, tag="outT", name=f"outT{_h}") for _h in range(HPC)]
        pending = []  # [(g, [pt_h0, pt_h1])], PV lags 2 quads behind QK/exp
        udone = 0
        outT_c = outT  # capture for the deferred normalize

        def flush_pv(g, pts):
            for h in range(HPC):
                for i in range(2):
                    kb = 2 * g + i
                    off = max(0, kb * KB - c * SPAN)
                    nc.tensor.matmul(outT[h][:, off:SPAN],
                                     vsb[:, kb, h, 0:65],
                                     pts[h][:, i * SPAN + off:(i + 1) * SPAN],
                                     start=(kb == 0), stop=(kb == nkb - 1))

        for g in range(ngrp):
            pts = []
            qps_l = []
            for h in range(HPC):
                qps = scoresp.tile([128, 2 * SPAN], F32, tag="sc")
                for i in range(2):
                    kb = 2 * g + i
                    nc.tensor.matmul(
                        qps[:, i * SPAN:(i + 1) * SPAN],
                        kT[h * HD:(h + 1) * HD, kb * KB:(kb + 1) * KB],
                        qT[h * HD:(h + 1) * HD, c * SPAN:(c + 1) * SPAN],
                        start=True, stop=True)
                qps_l.append(qps)
            for h in range(HPC):
                qps = qps_l[h]
                for i in range(2):
                    kb = 2 * g + i
                    off = kb * KB - c * SPAN
                    if off >= 0:  # diagonal block: triangle mask on its 128 cols
                        nc.vector.tensor_add(
                            out=qps[:, i * SPAN + off:i * SPAN + off + KB],
                            in0=qps[:, i * SPAN + off:i * SPAN + off + KB],
                            in1=mbig_sb[:, 384:512])
                pt = ptp.tile([128, 2 * SPAN], F32R, tag="pt")
                nc.scalar.activation(out=pt, in_=qps,
                                     func=mybir.ActivationFunctionType.Exp,
                                     scale=float(1.0 / np.sqrt(HD)))
                pts.append(pt)
            if g == 0 and norm_thunks:
                # previous span's normalize, overlapped into this span's pipe
                norm_thunks.pop(0)()
            pending.append((g, pts))
            if len(pending) > 2:
                flush_pv(*pending.pop(0))
            # sprinkle qkv units (reserve a share for the span tail) and
            # drain roughly one deferred wo unit per quad
            target = ((g + 1) * len(units)) // (ngrp + 2)
            while udone < target:
                units[udone]()
                udone += 1
            if wo_fifo:
                wo_fifo.pop(0)()
            if len(wo_fifo) > 10:
                wo_fifo.pop(0)()
        for k, item in enumerate(pending):
            flush_pv(*item)
            target = ((ngrp + 1 + k) * len(units)) // (ngrp + 2)
            while udone < target:
                units[udone]()
                udone += 1
            if wo_fifo:
                wo_fifo.pop(0)()

        while udone < len(units):
            units[udone]()
            udone += 1
        norm_thunks.append(make_normalize(c, outT_c))

    # ---- epilogue: last normalize + remaining wo units ----
    for t in norm_thunks:
        t()
    norm_thunks.clear()
    while wo_fifo:
        wo_fifo.pop(0)()




# ---------------------------------------------------------------- host side
def _prep_core_inputs(r, xTf, Wqkv_w, Wqkv_b, Wo_w):
    g0, g1 = HPC * r, HPC * r + 1
    Wq, Wk, Wv = Wqkv_w[0:C], Wqkv_w[C:2 * C], Wqkv_w[2 * C:3 * C]
    bq, bk, bvv = Wqkv_b[0:C], Wqkv_b[C:2 * C], Wqkv_b[2 * C:3 * C]
    rows0 = slice(HD * g0, HD * g0 + HD)
    rows1 = slice(HD * g1, HD * g1 + HD)
    wqkv = np.concatenate(
        [Wq[rows0].T, Wq[rows1].T, Wk[rows0].T, Wk[rows1].T,
         Wv[rows0].T, Wv[rows1].T], axis=1)
    bqkv = np.stack(
        [np.concatenate([bq[rows0], bq[rows1]]),
         np.concatenate([bk[rows0], bk[rows1]]),
         np.concatenate([bvv[rows0], bvv[rows1]])], axis=1)
    wo = np.concatenate([Wo_w[:, rows0], Wo_w[:, rows1]], axis=1).T
    return {
        "xT": np.ascontiguousarray(xTf),
        "wqkv": np.ascontiguousarray(wqkv, np.float32),
        "wo": np.ascontiguousarray(wo, np.float32),
        "bqkv": np.ascontiguousarray(bqkv, np.float32),
        "mbig": _mbig(),
        "onesd": np.ones((1, 65), np.float32),
        "eye": np.eye(128, dtype=np.float32),
    }


def _mbig():
    m = np.full((KB, 896), NEG, np.float32)
    i = np.arange(KB)[:, None]
    cidx = np.arange(896)[None, :]
    m[cidx >= i + 384] = 0.0
    return m


def make_in_maps(x, Wqkv_w, Wqkv_b, Wo_w):
    xTf = np.ascontiguousarray(np.asarray(x, np.float32)[0].T)
    return [_prep_core_inputs(r, xTf, np.asarray(Wqkv_w, np.float32),
                              np.asarray(Wqkv_b, np.float32),
                              np.asarray(Wo_w, np.float32))
            for r in range(NCORES)]


_NC_CACHE = {}


def kernel(x, mask, Wqkv_w, Wqkv_b, Wo_w, Wo_b):
    from concourse.bass_utils import run_bass_kernel_spmd
    # The padding mask is all-False for this problem (spec fill=zeros);
    # causal masking is handled on-device.
    if 1 not in _NC_CACHE:
        _NC_CACHE[1] = build_nc(1)
    nc = _NC_CACHE[1]
    in_maps = make_in_maps(x, Wqkv_w, Wqkv_b, Wo_w)
    res = run_bass_kernel_spmd(nc, in_maps, core_ids=list(range(NCORES)))
    out = np.zeros((S, C), np.float64)
    for r in range(NCORES):
        out += res.results[r]["y"].astype(np.float64)
    out += np.asarray(Wo_b, np.float32).astype(np.float64)
    return out.astype(np.float32)[None, :, :]
